# Initial kernel scaffold
#
"""2-layer GAT (GATConv x2 + linear classifier) on 8 Trainium2 NeuronCores.

Self-contained kernel: takes full unsharded inputs, shards internally
(nodes in 8 contiguous slabs, edges owned by destination core), runs a
single SPMD NEFF on cores 0-7 with AllGather collectives between layers,
and returns (logits [N,16], x [N,64]) as float32 — matching the reference.

Per-core on-device flow:
  L1: slab1[n] = [h1(64) | as1 | ad1 | 1 | pad]   (h1 = x @ W1, node matmul)
      AllGather -> table1 [N, 68] in shared DRAM
      per 128-dst super-block, per 128-edge chunk:
        rows = table1[src[e]]                       (SWDGE indirect DMA gather)
        S[e, m] = exp(lrelu(as[src]+ad[dst])) * (dst_rel[e] == m)
        psum[128 dst, 67] += S.T @ rows[:, 0:67]    (col 66 = ones -> sum p)
      x2 = elu(agg / sum_p + b1);  as2/ad2 row-dots -> slab2; AllGather
  L2: same aggregation over [x2 | as2 | ad2 | 1]; then
      [x_out | logits] = (agg2 / sum_p2) @ [W2 | W2@cW] + [b2 | b2@cW+cb]
      (W2 commutes with the attention-weighted sum, so layer 2 gathers raw
       x2 rows and applies W2 and the classifier after aggregation.)
Softmax max-subtraction is dropped: logits are bounded (~±30), exp is safe
in f32 and softmax is shift-invariant.
"""
import numpy as np

P = 128
NC = 8
IN_DIM = 128
H = 64
NCLS = 16
F = 68
SUP = 128
FM = 67
N_NODES = 100000
N_EDGES = 1600000


def _install_axon_shim():
    """antenv.axon_hooks is absent from this image; bass_utils imports it
    unconditionally when tracing. Provide a functional stand-in."""
    import sys
    import types
    if 'antenv.axon_hooks' in sys.modules:
        return
    mod = types.ModuleType('antenv.axon_hooks')
    mod._hook = None
    mod.set_axon_ntff_profile_hook = lambda h: setattr(mod, '_hook', h)
    mod.get_axon_ntff_profile_hook = lambda: mod._hook
    sys.modules['antenv.axon_hooks'] = mod
    try:
        import antenv
        antenv.axon_hooks = mod
        from trn_agent_boot.trn_boot import _ntff_profile_via_ctypes
        mod._hook = _ntff_profile_via_ctypes('/opt/axon/libaxon_pjrt.so')
    except Exception:
        pass
    try:
        import concourse.bass_utils as bu
        bu.upload_artifacts = lambda tmpdir: f"file://{tmpdir}"
    except Exception:
        pass


def _preprocess(ft, adj, W1, a_s1, a_d1, b1, W2, a_s2, a_d2, b2, cW, cb):
    N = ft.shape[0]
    NOWN = N // NC
    loop = np.arange(N, dtype=np.int64)
    src = np.concatenate([adj[0], loop]).astype(np.int64)
    dst = np.concatenate([adj[1], loop]).astype(np.int64)

    NSUP = (NOWN + SUP - 1) // SUP
    sup_sizes = [min(SUP, NOWN - s * SUP) for s in range(NSUP)]

    owner = dst // NOWN
    cores = []
    for c in range(NC):
        m = owner == c
        s_c = src[m]
        d_c = dst[m] - c * NOWN
        order = np.argsort(d_c, kind="stable")
        cores.append((s_c[order], d_c[order]))

    K = np.zeros(NSUP, np.int64)
    edge_bounds = []
    for c in range(NC):
        d_c = cores[c][1]
        sup_id = d_c // SUP
        counts = np.bincount(sup_id, minlength=NSUP)
        bounds = np.zeros(NSUP + 1, np.int64)
        np.cumsum(counts, out=bounds[1:])
        edge_bounds.append(bounds)
        K = np.maximum(K, (counts + P - 1) // P)
    K = np.maximum(K, 1).astype(np.int64)
    NCHUNK = int(K.sum())
    chunk_off = np.concatenate([[0], np.cumsum(K)])

    w1ext = np.zeros((IN_DIM, 66), np.float32)
    w1ext[:, 0:64] = W1
    w1ext[:, 64] = W1 @ a_s1
    w1ext[:, 65] = W1 @ a_d1
    w2s = (W2 @ a_s2).astype(np.float32)
    w2d = (W2 @ a_d2).astype(np.float32)
    wfin = np.concatenate([W2, W2 @ cW], axis=1).astype(np.float32)
    bfin = np.concatenate([b2, (b2 @ cW + cb[0])]).astype(np.float32)

    per_core = []
    for c in range(NC):
        s_c, d_c = cores[c]
        bounds = edge_bounds[c]
        src_arr = np.zeros((NCHUNK, P), np.int32)
        rel_arr = np.full((NCHUNK, P), 999.0, np.float32)
        for s in range(NSUP):
            e0, e1 = bounds[s], bounds[s + 1]
            n_e = e1 - e0
            c0 = chunk_off[s]
            slots_src = np.zeros(int(K[s]) * P, np.int32)
            slots_rel = np.full(int(K[s]) * P, 999.0, np.float32)
            slots_src[:n_e] = s_c[e0:e1]
            slots_rel[:n_e] = (d_c[e0:e1] - s * SUP).astype(np.float32)
            src_arr[c0:c0 + K[s]] = slots_src.reshape(int(K[s]), P)
            rel_arr[c0:c0 + K[s]] = slots_rel.reshape(int(K[s]), P)
        per_core.append({
            "xT_own": np.ascontiguousarray(ft[c * NOWN:(c + 1) * NOWN].T).astype(np.float32),
            "w1ext": w1ext,
            "wfin": wfin,
            "bfin_rep": np.tile(bfin, (P, 1)),
            "b1_rep": np.tile(b1.astype(np.float32), (P, 1)),
            "w2s_rep": np.tile(w2s, (P, 1)),
            "w2d_rep": np.tile(w2d, (P, 1)),
            "iota128": np.tile(np.arange(P, dtype=np.float32), (P, 1)),
            "ident": np.eye(P, dtype=np.float32),
            "src_all": np.ascontiguousarray(src_arr.T),
            "rel_all": np.ascontiguousarray(rel_arr.T),
        })

    meta = dict(N=N, NOWN=NOWN, NSUP=NSUP, sup_sizes=sup_sizes,
                K=[int(k) for k in K], NCHUNK=NCHUNK)
    return meta, per_core


def _edge_phase(nc, meta, table, ad_dram, iota128, src_in, rel_in, layer,
                slab_out, ad_out, b1_rep, w2s_rep, w2d_rep,
                ident_in, wfin, bfin_rep, x_out, logits_out):
    import concourse.bass as bass
    import concourse.mybir as mybir
    import concourse.tile as tile

    NOWN = meta["NOWN"]; NSUP = meta["NSUP"]
    sup_sizes = meta["sup_sizes"]; K = meta["K"]; NCHUNK = meta["NCHUNK"]
    f32 = mybir.dt.float32

    with tile.TileContext(nc) as tc:
        with (
            tc.tile_pool(name=f"e{layer}c", bufs=1) as cpool,
            tc.tile_pool(name=f"e{layer}g", bufs=12) as gpool,
            tc.tile_pool(name=f"e{layer}s", bufs=6) as spool,
            tc.tile_pool(name=f"e{layer}w", bufs=4) as wpool,
            tc.tile_pool(name=f"e{layer}ps", bufs=3, space="PSUM") as pspool,
            tc.tile_pool(name=f"e{layer}ps2", bufs=2, space="PSUM") as pspool2,
        ):
            iota_t = cpool.tile([P, P], f32)
            nc.sync.dma_start(out=iota_t[:], in_=iota128[:])
            src_t = cpool.tile([P, NCHUNK], mybir.dt.int32)
            nc.sync.dma_start(out=src_t[:], in_=src_in[:])
            rel_t = cpool.tile([P, NCHUNK], f32)
            nc.sync.dma_start(out=rel_t[:], in_=rel_in[:])
            NPAD = NSUP * SUP
            ad_all = cpool.tile([P, NPAD], f32)
            if NPAD > NOWN:
                nc.vector.memset(ad_all[:, NOWN:NPAD], 0.0)
            nc.sync.dma_start(out=ad_all[:, :NOWN],
                              in_=ad_dram[0:1, :].partition_broadcast(P))
            if layer == 1:
                b1_t = cpool.tile([P, H], f32)
                nc.sync.dma_start(out=b1_t[:], in_=b1_rep[:])
                w2s_t = cpool.tile([P, H], f32)
                nc.sync.dma_start(out=w2s_t[:], in_=w2s_rep[:])
                w2d_t = cpool.tile([P, H], f32)
                nc.sync.dma_start(out=w2d_t[:], in_=w2d_rep[:])
            else:
                ident_t = cpool.tile([P, P], f32)
                nc.sync.dma_start(out=ident_t[:], in_=ident_in[:])
                wfin_t = cpool.tile([H, 80], f32)
                nc.sync.dma_start(out=wfin_t[:], in_=wfin[:])
                bfin_t = cpool.tile([P, 80], f32)
                nc.sync.dma_start(out=bfin_t[:], in_=bfin_rep[:])

            cix = 0
            for s in range(NSUP):
                sz = sup_sizes[s]
                Ks = K[s]
                ps = pspool.tile([P, F], f32, tag="ps")
                for k in range(Ks):
                    c = cix + k
                    g = gpool.tile([P, F], f32, tag="g")
                    nc.gpsimd.indirect_dma_start(
                        out=g[:], out_offset=None, in_=table[:],
                        in_offset=bass.IndirectOffsetOnAxis(
                            ap=src_t[:, c:c + 1], axis=0),
                    )
                    mask = spool.tile([P, P], f32, tag="mask")
                    nc.vector.tensor_scalar(
                        out=mask[:], in0=iota_t[:], scalar1=rel_t[:, c:c + 1],
                        scalar2=None, op0=mybir.AluOpType.is_equal)
                    tmp = spool.tile([P, P], f32, tag="tmp")
                    nc.vector.tensor_tensor(
                        out=tmp[:], in0=mask[:],
                        in1=ad_all[:, s * SUP:s * SUP + P],
                        op=mybir.AluOpType.mult)
                    z = spool.tile([P, 1], f32, tag="z")
                    nc.vector.tensor_reduce(
                        out=z[:], in_=tmp[:], axis=mybir.AxisListType.X,
                        op=mybir.AluOpType.add)
                    nc.vector.tensor_tensor(out=z[:], in0=z[:], in1=g[:, 64:65],
                                            op=mybir.AluOpType.add)
                    zs = spool.tile([P, 1], f32, tag="zs")
                    nc.vector.tensor_scalar(out=zs[:], in0=z[:], scalar1=0.2,
                                            scalar2=None, op0=mybir.AluOpType.mult)
                    nc.vector.tensor_tensor(out=z[:], in0=z[:], in1=zs[:],
                                            op=mybir.AluOpType.max)
                    p_t = spool.tile([P, 1], f32, tag="p_t")
                    nc.scalar.activation(out=p_t[:], in_=z[:],
                                         func=mybir.ActivationFunctionType.Exp)
                    stile = spool.tile([P, P], f32, tag="stile")
                    nc.vector.tensor_scalar(
                        out=stile[:], in0=mask[:], scalar1=p_t[:, 0:1],
                        scalar2=None, op0=mybir.AluOpType.mult)
                    nc.tensor.matmul(out=ps[:, 0:FM], lhsT=stile[:],
                                     rhs=g[:, 0:FM],
                                     start=(k == 0), stop=(k == Ks - 1))
                cix += Ks

                rcp = wpool.tile([P, 1], f32, tag="rcp")
                nc.vector.reciprocal(out=rcp[:sz], in_=ps[:sz, 66:67])
                if layer == 1:
                    st = wpool.tile([P, F], f32, tag="st")
                    t0 = wpool.tile([P, H], f32, tag="t0")
                    nc.vector.tensor_scalar(out=t0[:sz], in0=ps[:sz, 0:H],
                                            scalar1=rcp[:sz, 0:1], scalar2=None,
                                            op0=mybir.AluOpType.mult)
                    nc.vector.tensor_tensor(out=t0[:sz], in0=t0[:sz],
                                            in1=b1_t[:sz], op=mybir.AluOpType.add)
                    m0 = wpool.tile([P, H], f32, tag="m0")
                    nc.vector.tensor_scalar(out=m0[:sz], in0=t0[:sz], scalar1=0.0,
                                            scalar2=None, op0=mybir.AluOpType.min)
                    nc.scalar.activation(out=m0[:sz], in_=m0[:sz],
                                         func=mybir.ActivationFunctionType.Exp)
                    nc.vector.tensor_scalar(out=m0[:sz], in0=m0[:sz], scalar1=-1.0,
                                            scalar2=None, op0=mybir.AluOpType.add)
                    nc.vector.tensor_scalar(out=t0[:sz], in0=t0[:sz], scalar1=0.0,
                                            scalar2=None, op0=mybir.AluOpType.max)
                    nc.vector.tensor_tensor(out=st[:sz, 0:H], in0=m0[:sz],
                                            in1=t0[:sz], op=mybir.AluOpType.add)
                    td = wpool.tile([P, H], f32, tag="td")
                    nc.vector.tensor_tensor(out=td[:sz], in0=st[:sz, 0:H],
                                            in1=w2s_t[:sz], op=mybir.AluOpType.mult)
                    nc.vector.tensor_reduce(out=st[:sz, 64:65], in_=td[:sz],
                                            axis=mybir.AxisListType.X,
                                            op=mybir.AluOpType.add)
                    nc.vector.tensor_tensor(out=td[:sz], in0=st[:sz, 0:H],
                                            in1=w2d_t[:sz], op=mybir.AluOpType.mult)
                    nc.vector.tensor_reduce(out=st[:sz, 65:66], in_=td[:sz],
                                            axis=mybir.AxisListType.X,
                                            op=mybir.AluOpType.add)
                    nc.vector.memset(st[:sz, 66:68], 1.0)
                    nc.sync.dma_start(out=slab_out[s * SUP:s * SUP + sz, :],
                                      in_=st[:sz, :])
                    nc.sync.dma_start(
                        out=ad_out[0:1, s * SUP:s * SUP + sz].rearrange("one n -> n one"),
                        in_=st[:sz, 65:66])
                else:
                    a2 = wpool.tile([P, H], f32, tag="a2")
                    nc.vector.tensor_scalar(out=a2[:sz], in0=ps[:sz, 0:H],
                                            scalar1=rcp[:sz, 0:1], scalar2=None,
                                            op0=mybir.AluOpType.mult)
                    psT = pspool2.tile([H, P], f32, tag="psT")
                    nc.tensor.transpose(out=psT[:, :sz], in_=a2[:sz],
                                        identity=ident_t[:])
                    a2T = wpool.tile([H, P], f32, tag="a2T")
                    nc.vector.tensor_copy(out=a2T[:, :sz], in_=psT[:, :sz])
                    psF = pspool2.tile([P, 80], f32, tag="psF")
                    nc.tensor.matmul(out=psF[:sz], lhsT=a2T[:, :sz], rhs=wfin_t[:],
                                     start=True, stop=True)
                    fin = wpool.tile([P, 80], f32, tag="fin")
                    nc.vector.tensor_tensor(out=fin[:sz], in0=psF[:sz],
                                            in1=bfin_t[:sz], op=mybir.AluOpType.add)
                    nc.sync.dma_start(out=x_out[s * SUP:s * SUP + sz, :],
                                      in_=fin[:sz, 0:H])
                    nc.sync.dma_start(out=logits_out[s * SUP:s * SUP + sz, :],
                                      in_=fin[:sz, H:80])


def _build(meta):
    import concourse.bacc as bacc
    import concourse.mybir as mybir
    import concourse.tile as tile

    N = meta["N"]; NOWN = meta["NOWN"]; NCHUNK = meta["NCHUNK"]

    nc = bacc.Bacc("TRN2", target_bir_lowering=True)
    f32 = mybir.dt.float32

    xT_own = nc.dram_tensor("xT_own", [IN_DIM, NOWN], f32, kind="ExternalInput")
    w1ext = nc.dram_tensor("w1ext", [IN_DIM, 66], f32, kind="ExternalInput")
    wfin = nc.dram_tensor("wfin", [H, 80], f32, kind="ExternalInput")
    bfin_rep = nc.dram_tensor("bfin_rep", [P, 80], f32, kind="ExternalInput")
    b1_rep = nc.dram_tensor("b1_rep", [P, H], f32, kind="ExternalInput")
    w2s_rep = nc.dram_tensor("w2s_rep", [P, H], f32, kind="ExternalInput")
    w2d_rep = nc.dram_tensor("w2d_rep", [P, H], f32, kind="ExternalInput")
    iota128 = nc.dram_tensor("iota128", [P, P], f32, kind="ExternalInput")
    ident_in = nc.dram_tensor("ident", [P, P], f32, kind="ExternalInput")
    src_in = nc.dram_tensor("src_all", [P, NCHUNK], mybir.dt.int32, kind="ExternalInput")
    rel_in = nc.dram_tensor("rel_all", [P, NCHUNK], f32, kind="ExternalInput")

    x_out = nc.dram_tensor("x_out", [NOWN, H], f32, kind="ExternalOutput")
    logits_out = nc.dram_tensor("logits_out", [NOWN, NCLS], f32, kind="ExternalOutput")

    slab1 = nc.dram_tensor("slab1", [NOWN, F], f32)
    slab2 = nc.dram_tensor("slab2", [NOWN, F], f32)
    table1 = nc.dram_tensor("table1", [N, F], f32, addr_space="Shared")
    table2 = nc.dram_tensor("table2", [N, F], f32, addr_space="Shared")
    ad1_dram = nc.dram_tensor("ad1_dram", [1, NOWN], f32)
    ad2_dram = nc.dram_tensor("ad2_dram", [1, NOWN], f32)

    rg = [list(range(NC))]

    with tile.TileContext(nc) as tc:
        with (
            tc.tile_pool(name="p1c", bufs=1) as cpool,
            tc.tile_pool(name="p1w", bufs=3) as wpool,
            tc.tile_pool(name="p1ps", bufs=3, space="PSUM") as pspool,
        ):
            w1t = cpool.tile([IN_DIM, 66], f32)
            nc.sync.dma_start(out=w1t[:], in_=w1ext[:])
            GN = 512
            n_grp = (NOWN + GN - 1) // GN
            for t in range(n_grp):
                n0 = t * GN
                gn = min(GN, NOWN - n0)
                xt = wpool.tile([IN_DIM, GN], f32, tag="xt")
                nc.sync.dma_start(out=xt[:, :gn], in_=xT_own[:, n0:n0 + gn])
                psa = pspool.tile([1, GN], f32, tag="psa")
                nc.tensor.matmul(out=psa[:, :gn], lhsT=w1t[:, 65:66], rhs=xt[:, :gn],
                                 start=True, stop=True)
                adrow = wpool.tile([1, GN], f32, tag="adrow")
                nc.vector.tensor_copy(out=adrow[:, :gn], in_=psa[:, :gn])
                nc.sync.dma_start(out=ad1_dram[0:1, n0:n0 + gn], in_=adrow[:, :gn])
                for j in range((gn + P - 1) // P):
                    m0 = j * P
                    mn = min(P, gn - m0)
                    ps = pspool.tile([P, F], f32, tag="ps")
                    nc.tensor.matmul(out=ps[:mn, 0:66], lhsT=xt[:, m0:m0 + mn],
                                     rhs=w1t[:], start=True, stop=True)
                    st = wpool.tile([P, F], f32, tag="st")
                    nc.vector.tensor_copy(out=st[:mn, 0:66], in_=ps[:mn, 0:66])
                    nc.vector.memset(st[:mn, 66:68], 1.0)
                    nc.sync.dma_start(out=slab1[n0 + m0:n0 + m0 + mn, :],
                                      in_=st[:mn, :])

    cc1 = nc.alloc_semaphore("cc1")
    nc.gpsimd.collective_compute(
        "AllGather", mybir.AluOpType.bypass, replica_groups=rg,
        ins=[slab1.ap().opt()], outs=[table1.ap().opt()],
    ).then_inc(cc1, 1)
    nc.gpsimd.wait_ge(cc1, 1)

    _edge_phase(nc, meta, table1, ad1_dram, iota128, src_in, rel_in,
                layer=1, slab_out=slab2, ad_out=ad2_dram,
                b1_rep=b1_rep, w2s_rep=w2s_rep, w2d_rep=w2d_rep,
                ident_in=None, wfin=None, bfin_rep=None,
                x_out=None, logits_out=None)

    cc2 = nc.alloc_semaphore("cc2")
    nc.gpsimd.collective_compute(
        "AllGather", mybir.AluOpType.bypass, replica_groups=rg,
        ins=[slab2.ap().opt()], outs=[table2.ap().opt()],
    ).then_inc(cc2, 1)
    nc.gpsimd.wait_ge(cc2, 1)

    _edge_phase(nc, meta, table2, ad2_dram, iota128, src_in, rel_in,
                layer=2, slab_out=None, ad_out=None,
                b1_rep=None, w2s_rep=None, w2d_rep=None,
                ident_in=ident_in, wfin=wfin, bfin_rep=bfin_rep,
                x_out=x_out, logits_out=logits_out)

    nc.compile()
    return nc


def kernel(**inputs):
    _install_axon_shim()
    from concourse.bass_utils import run_bass_kernel_spmd

    ft = np.asarray(inputs["ft_list"], dtype=np.float32)
    adj = np.asarray(inputs["adj_tensor"])
    args = [ft, adj] + [np.asarray(inputs[k], dtype=np.float32) for k in
                        ("W1", "a_src1", "a_dst1", "b1",
                         "W2", "a_src2", "a_dst2", "b2", "clas_W", "clas_b")]
    meta, per_core = _preprocess(*args)
    nc = _build(meta)
    res = run_bass_kernel_spmd(nc, per_core, core_ids=list(range(NC)))
    logits = np.concatenate([res.results[c]["logits_out"] for c in range(NC)], axis=0)
    x = np.concatenate([res.results[c]["x_out"] for c in range(NC)], axis=0)
    return logits.astype(np.float32), x.astype(np.float32)


# revision 1
# speedup vs baseline: 1.0115x; 1.0115x over previous
"""2-layer GAT (GATConv x2 + linear classifier) on 8 Trainium2 NeuronCores.

Self-contained kernel: takes full unsharded inputs, shards internally
(nodes in 8 contiguous slabs, edges owned by destination core), runs a
single SPMD NEFF on cores 0-7 with AllGather collectives between layers,
and returns (logits [N,16], x [N,64]) as float32 — matching the reference.

Per-core on-device flow:
  L1: slab1[n] = [h1(64) | as1 | ad1 | 1 | pad]   (h1 = x @ W1, node matmul)
      AllGather -> table1 [N, 68] in shared DRAM
      per 128-dst super-block, per 128-edge chunk:
        rows = table1[src[e]]                       (SWDGE indirect DMA gather)
        S[e, m] = exp(lrelu(as[src]+ad[dst])) * (dst_rel[e] == m)
        psum[128 dst, 67] += S.T @ rows[:, 0:67]    (col 66 = ones -> sum p)
      x2 = elu(agg / sum_p + b1);  as2/ad2 row-dots -> slab2; AllGather
  L2: same aggregation over [x2 | as2 | ad2 | 1]; then
      [x_out | logits] = (agg2 / sum_p2) @ [W2 | W2@cW] + [b2 | b2@cW+cb]
      (W2 commutes with the attention-weighted sum, so layer 2 gathers raw
       x2 rows and applies W2 and the classifier after aggregation.)
Softmax max-subtraction is dropped: logits are bounded (~±30), exp is safe
in f32 and softmax is shift-invariant.
"""
import numpy as np

P = 128
NC = 8
IN_DIM = 128
H = 64
NCLS = 16
F = 68
SUP = 128
FM = 67
N_NODES = 100000
N_EDGES = 1600000


def _install_axon_shim():
    """antenv.axon_hooks is absent from this image; bass_utils imports it
    unconditionally when tracing. Provide a functional stand-in."""
    import sys
    import types
    if 'antenv.axon_hooks' in sys.modules:
        return
    mod = types.ModuleType('antenv.axon_hooks')
    mod._hook = None
    mod.set_axon_ntff_profile_hook = lambda h: setattr(mod, '_hook', h)
    mod.get_axon_ntff_profile_hook = lambda: mod._hook
    sys.modules['antenv.axon_hooks'] = mod
    try:
        import antenv
        antenv.axon_hooks = mod
        from trn_agent_boot.trn_boot import _ntff_profile_via_ctypes
        mod._hook = _ntff_profile_via_ctypes('/opt/axon/libaxon_pjrt.so')
    except Exception:
        pass
    try:
        import concourse.bass_utils as bu
        bu.upload_artifacts = lambda tmpdir: f"file://{tmpdir}"
    except Exception:
        pass


def _preprocess(ft, adj, W1, a_s1, a_d1, b1, W2, a_s2, a_d2, b2, cW, cb):
    N = ft.shape[0]
    NOWN = N // NC
    loop = np.arange(N, dtype=np.int64)
    src = np.concatenate([adj[0], loop]).astype(np.int64)
    dst = np.concatenate([adj[1], loop]).astype(np.int64)

    NSUP = (NOWN + SUP - 1) // SUP
    sup_sizes = [min(SUP, NOWN - s * SUP) for s in range(NSUP)]

    owner = dst // NOWN
    cores = []
    for c in range(NC):
        m = owner == c
        s_c = src[m]
        d_c = dst[m] - c * NOWN
        order = np.argsort(d_c, kind="stable")
        cores.append((s_c[order], d_c[order]))

    K = np.zeros(NSUP, np.int64)
    edge_bounds = []
    for c in range(NC):
        d_c = cores[c][1]
        sup_id = d_c // SUP
        counts = np.bincount(sup_id, minlength=NSUP)
        bounds = np.zeros(NSUP + 1, np.int64)
        np.cumsum(counts, out=bounds[1:])
        edge_bounds.append(bounds)
        K = np.maximum(K, (counts + P - 1) // P)
    K = np.maximum(K, 1).astype(np.int64)
    NCHUNK = int(K.sum())
    chunk_off = np.concatenate([[0], np.cumsum(K)])

    w1ext = np.zeros((IN_DIM, 66), np.float32)
    w1ext[:, 0:64] = W1
    w1ext[:, 64] = W1 @ a_s1
    w1ext[:, 65] = W1 @ a_d1
    w2s = (W2 @ a_s2).astype(np.float32)
    w2d = (W2 @ a_d2).astype(np.float32)
    wfin = np.concatenate([W2, W2 @ cW], axis=1).astype(np.float32)
    bfin = np.concatenate([b2, (b2 @ cW + cb[0])]).astype(np.float32)

    per_core = []
    for c in range(NC):
        s_c, d_c = cores[c]
        bounds = edge_bounds[c]
        src_arr = np.zeros((NCHUNK, P), np.int32)
        rel_arr = np.full((NCHUNK, P), 999.0, np.float32)
        for s in range(NSUP):
            e0, e1 = bounds[s], bounds[s + 1]
            n_e = e1 - e0
            c0 = chunk_off[s]
            slots_src = np.zeros(int(K[s]) * P, np.int32)
            slots_rel = np.full(int(K[s]) * P, 999.0, np.float32)
            slots_src[:n_e] = s_c[e0:e1]
            slots_rel[:n_e] = (d_c[e0:e1] - s * SUP).astype(np.float32)
            src_arr[c0:c0 + K[s]] = slots_src.reshape(int(K[s]), P)
            rel_arr[c0:c0 + K[s]] = slots_rel.reshape(int(K[s]), P)
        per_core.append({
            "xT_own": np.ascontiguousarray(ft[c * NOWN:(c + 1) * NOWN].T).astype(np.float32),
            "w1ext": w1ext,
            "wfin": wfin,
            "bfin_rep": np.tile(bfin, (P, 1)),
            "b1_rep": np.tile(b1.astype(np.float32), (P, 1)),
            "w2s_rep": np.tile(w2s, (P, 1)),
            "w2d_rep": np.tile(w2d, (P, 1)),
            "iota128": np.tile(np.arange(P, dtype=np.float32), (P, 1)),
            "ident": np.eye(P, dtype=np.float32),
            "src_all": np.ascontiguousarray(src_arr.T),
            "rel_all": np.ascontiguousarray(rel_arr.T),
        })

    meta = dict(N=N, NOWN=NOWN, NSUP=NSUP, sup_sizes=sup_sizes,
                K=[int(k) for k in K], NCHUNK=NCHUNK)
    return meta, per_core


def _edge_phase(nc, meta, table, ad_dram, iota128, src_in, rel_in, layer,
                slab_out, ad_out, b1_rep, w2s_rep, w2d_rep,
                ident_in, wfin, bfin_rep, x_out, logits_out):
    import concourse.bass as bass
    import concourse.mybir as mybir
    import concourse.tile as tile

    NOWN = meta["NOWN"]; NSUP = meta["NSUP"]
    sup_sizes = meta["sup_sizes"]; K = meta["K"]; NCHUNK = meta["NCHUNK"]
    f32 = mybir.dt.float32

    with tile.TileContext(nc) as tc:
        with (
            tc.tile_pool(name=f"e{layer}c", bufs=1) as cpool,
            tc.tile_pool(name=f"e{layer}g", bufs=12) as gpool,
            tc.tile_pool(name=f"e{layer}s", bufs=6) as spool,
            tc.tile_pool(name=f"e{layer}w", bufs=4) as wpool,
            tc.tile_pool(name=f"e{layer}ps", bufs=3, space="PSUM") as pspool,
            tc.tile_pool(name=f"e{layer}ps2", bufs=2, space="PSUM") as pspool2,
        ):
            iota_t = cpool.tile([P, P], f32)
            nc.sync.dma_start(out=iota_t[:], in_=iota128[:])
            src_t = cpool.tile([P, NCHUNK], mybir.dt.int32)
            nc.sync.dma_start(out=src_t[:], in_=src_in[:])
            rel_t = cpool.tile([P, NCHUNK], f32)
            nc.sync.dma_start(out=rel_t[:], in_=rel_in[:])
            NPAD = NSUP * SUP
            ad_all = cpool.tile([P, NPAD], f32)
            if NPAD > NOWN:
                nc.vector.memset(ad_all[:, NOWN:NPAD], 0.0)
            nc.sync.dma_start(out=ad_all[:, :NOWN],
                              in_=ad_dram[0:1, :].partition_broadcast(P))
            if layer == 1:
                b1_t = cpool.tile([P, H], f32)
                nc.sync.dma_start(out=b1_t[:], in_=b1_rep[:])
                w2s_t = cpool.tile([P, H], f32)
                nc.sync.dma_start(out=w2s_t[:], in_=w2s_rep[:])
                w2d_t = cpool.tile([P, H], f32)
                nc.sync.dma_start(out=w2d_t[:], in_=w2d_rep[:])
            else:
                ident_t = cpool.tile([P, P], f32)
                nc.sync.dma_start(out=ident_t[:], in_=ident_in[:])
                wfin_t = cpool.tile([H, 80], f32)
                nc.sync.dma_start(out=wfin_t[:], in_=wfin[:])
                bfin_t = cpool.tile([P, 80], f32)
                nc.sync.dma_start(out=bfin_t[:], in_=bfin_rep[:])

            cix = 0
            for s in range(NSUP):
                sz = sup_sizes[s]
                Ks = K[s]
                ps = pspool.tile([P, F], f32, tag="ps")
                for k in range(Ks):
                    c = cix + k
                    g = gpool.tile([P, F], f32, tag="g")
                    nc.gpsimd.indirect_dma_start(
                        out=g[:], out_offset=None, in_=table[:],
                        in_offset=bass.IndirectOffsetOnAxis(
                            ap=src_t[:, c:c + 1], axis=0),
                    )
                    mask = spool.tile([P, P], f32, tag="mask")
                    nc.vector.tensor_scalar(
                        out=mask[:], in0=iota_t[:], scalar1=rel_t[:, c:c + 1],
                        scalar2=None, op0=mybir.AluOpType.is_equal)
                    tmp = spool.tile([P, P], f32, tag="tmp")
                    nc.vector.tensor_tensor(
                        out=tmp[:], in0=mask[:],
                        in1=ad_all[:, s * SUP:s * SUP + P],
                        op=mybir.AluOpType.mult)
                    z = spool.tile([P, 1], f32, tag="z")
                    nc.vector.tensor_reduce(
                        out=z[:], in_=tmp[:], axis=mybir.AxisListType.X,
                        op=mybir.AluOpType.add)
                    nc.vector.tensor_tensor(out=z[:], in0=z[:], in1=g[:, 64:65],
                                            op=mybir.AluOpType.add)
                    zs = spool.tile([P, 1], f32, tag="zs")
                    nc.vector.tensor_scalar(out=zs[:], in0=z[:], scalar1=0.2,
                                            scalar2=None, op0=mybir.AluOpType.mult)
                    nc.vector.tensor_tensor(out=z[:], in0=z[:], in1=zs[:],
                                            op=mybir.AluOpType.max)
                    p_t = spool.tile([P, 1], f32, tag="p_t")
                    nc.scalar.activation(out=p_t[:], in_=z[:],
                                         func=mybir.ActivationFunctionType.Exp)
                    stile = spool.tile([P, P], f32, tag="stile")
                    nc.vector.tensor_scalar(
                        out=stile[:], in0=mask[:], scalar1=p_t[:, 0:1],
                        scalar2=None, op0=mybir.AluOpType.mult)
                    nc.tensor.matmul(out=ps[:, 0:FM], lhsT=stile[:],
                                     rhs=g[:, 0:FM],
                                     start=(k == 0), stop=(k == Ks - 1))
                cix += Ks

                rcp = wpool.tile([P, 1], f32, tag="rcp")
                nc.vector.reciprocal(out=rcp[:sz], in_=ps[:sz, 66:67])
                if layer == 1:
                    st = wpool.tile([P, F], f32, tag="st")
                    t0 = wpool.tile([P, H], f32, tag="t0")
                    nc.vector.tensor_scalar(out=t0[:sz], in0=ps[:sz, 0:H],
                                            scalar1=rcp[:sz, 0:1], scalar2=None,
                                            op0=mybir.AluOpType.mult)
                    nc.vector.tensor_tensor(out=t0[:sz], in0=t0[:sz],
                                            in1=b1_t[:sz], op=mybir.AluOpType.add)
                    m0 = wpool.tile([P, H], f32, tag="m0")
                    nc.vector.tensor_scalar(out=m0[:sz], in0=t0[:sz], scalar1=0.0,
                                            scalar2=None, op0=mybir.AluOpType.min)
                    nc.scalar.activation(out=m0[:sz], in_=m0[:sz],
                                         func=mybir.ActivationFunctionType.Exp)
                    nc.vector.tensor_scalar(out=m0[:sz], in0=m0[:sz], scalar1=-1.0,
                                            scalar2=None, op0=mybir.AluOpType.add)
                    nc.vector.tensor_scalar(out=t0[:sz], in0=t0[:sz], scalar1=0.0,
                                            scalar2=None, op0=mybir.AluOpType.max)
                    nc.vector.tensor_tensor(out=st[:sz, 0:H], in0=m0[:sz],
                                            in1=t0[:sz], op=mybir.AluOpType.add)
                    td = wpool.tile([P, H], f32, tag="td")
                    nc.vector.tensor_tensor(out=td[:sz], in0=st[:sz, 0:H],
                                            in1=w2s_t[:sz], op=mybir.AluOpType.mult)
                    nc.vector.tensor_reduce(out=st[:sz, 64:65], in_=td[:sz],
                                            axis=mybir.AxisListType.X,
                                            op=mybir.AluOpType.add)
                    nc.vector.tensor_tensor(out=td[:sz], in0=st[:sz, 0:H],
                                            in1=w2d_t[:sz], op=mybir.AluOpType.mult)
                    nc.vector.tensor_reduce(out=st[:sz, 65:66], in_=td[:sz],
                                            axis=mybir.AxisListType.X,
                                            op=mybir.AluOpType.add)
                    nc.vector.memset(st[:sz, 66:68], 1.0)
                    nc.sync.dma_start(out=slab_out[s * SUP:s * SUP + sz, :],
                                      in_=st[:sz, :])
                    nc.sync.dma_start(
                        out=ad_out[0:1, s * SUP:s * SUP + sz].rearrange("one n -> n one"),
                        in_=st[:sz, 65:66])
                else:
                    a2 = wpool.tile([P, H], f32, tag="a2")
                    nc.vector.tensor_scalar(out=a2[:sz], in0=ps[:sz, 0:H],
                                            scalar1=rcp[:sz, 0:1], scalar2=None,
                                            op0=mybir.AluOpType.mult)
                    psT = pspool2.tile([H, P], f32, tag="psT")
                    nc.tensor.transpose(out=psT[:, :sz], in_=a2[:sz],
                                        identity=ident_t[:])
                    a2T = wpool.tile([H, P], f32, tag="a2T")
                    nc.vector.tensor_copy(out=a2T[:, :sz], in_=psT[:, :sz])
                    psF = pspool2.tile([P, 80], f32, tag="psF")
                    nc.tensor.matmul(out=psF[:sz], lhsT=a2T[:, :sz], rhs=wfin_t[:],
                                     start=True, stop=True)
                    fin = wpool.tile([P, 80], f32, tag="fin")
                    nc.vector.tensor_tensor(out=fin[:sz], in0=psF[:sz],
                                            in1=bfin_t[:sz], op=mybir.AluOpType.add)
                    nc.sync.dma_start(out=x_out[s * SUP:s * SUP + sz, :],
                                      in_=fin[:sz, 0:H])
                    nc.sync.dma_start(out=logits_out[s * SUP:s * SUP + sz, :],
                                      in_=fin[:sz, H:80])


def _build(meta):
    import concourse.bacc as bacc
    import concourse.mybir as mybir
    import concourse.tile as tile

    N = meta["N"]; NOWN = meta["NOWN"]; NCHUNK = meta["NCHUNK"]

    nc = bacc.Bacc("TRN2", target_bir_lowering=True)
    f32 = mybir.dt.float32

    xT_own = nc.dram_tensor("xT_own", [IN_DIM, NOWN], f32, kind="ExternalInput")
    w1ext = nc.dram_tensor("w1ext", [IN_DIM, 66], f32, kind="ExternalInput")
    wfin = nc.dram_tensor("wfin", [H, 80], f32, kind="ExternalInput")
    bfin_rep = nc.dram_tensor("bfin_rep", [P, 80], f32, kind="ExternalInput")
    b1_rep = nc.dram_tensor("b1_rep", [P, H], f32, kind="ExternalInput")
    w2s_rep = nc.dram_tensor("w2s_rep", [P, H], f32, kind="ExternalInput")
    w2d_rep = nc.dram_tensor("w2d_rep", [P, H], f32, kind="ExternalInput")
    iota128 = nc.dram_tensor("iota128", [P, P], f32, kind="ExternalInput")
    ident_in = nc.dram_tensor("ident", [P, P], f32, kind="ExternalInput")
    src_in = nc.dram_tensor("src_all", [P, NCHUNK], mybir.dt.int32, kind="ExternalInput")
    rel_in = nc.dram_tensor("rel_all", [P, NCHUNK], f32, kind="ExternalInput")

    x_out = nc.dram_tensor("x_out", [NOWN, H], f32, kind="ExternalOutput")
    logits_out = nc.dram_tensor("logits_out", [NOWN, NCLS], f32, kind="ExternalOutput")

    slab1 = nc.dram_tensor("slab1", [NOWN, F], f32)
    slab2 = nc.dram_tensor("slab2", [NOWN, F], f32)
    table1 = nc.dram_tensor("table1", [N, F], f32, addr_space="Shared")
    table2 = nc.dram_tensor("table2", [N, F], f32, addr_space="Shared")
    ad1_dram = nc.dram_tensor("ad1_dram", [1, NOWN], f32)
    ad2_dram = nc.dram_tensor("ad2_dram", [1, NOWN], f32)

    rg = [list(range(NC))]

    with tile.TileContext(nc) as tc:
        with (
            tc.tile_pool(name="p1c", bufs=1) as cpool,
            tc.tile_pool(name="p1w", bufs=3) as wpool,
            tc.tile_pool(name="p1ps", bufs=3, space="PSUM") as pspool,
        ):
            w1t = cpool.tile([IN_DIM, 66], f32)
            nc.sync.dma_start(out=w1t[:], in_=w1ext[:])
            GN = 512
            n_grp = (NOWN + GN - 1) // GN
            for t in range(n_grp):
                n0 = t * GN
                gn = min(GN, NOWN - n0)
                xt = wpool.tile([IN_DIM, GN], f32, tag="xt")
                nc.sync.dma_start(out=xt[:, :gn], in_=xT_own[:, n0:n0 + gn])
                psa = pspool.tile([1, GN], f32, tag="psa")
                nc.tensor.matmul(out=psa[:, :gn], lhsT=w1t[:, 65:66], rhs=xt[:, :gn],
                                 start=True, stop=True)
                adrow = wpool.tile([1, GN], f32, tag="adrow")
                nc.vector.tensor_copy(out=adrow[:, :gn], in_=psa[:, :gn])
                nc.sync.dma_start(out=ad1_dram[0:1, n0:n0 + gn], in_=adrow[:, :gn])
                for j in range((gn + P - 1) // P):
                    m0 = j * P
                    mn = min(P, gn - m0)
                    ps = pspool.tile([P, F], f32, tag="ps")
                    nc.tensor.matmul(out=ps[:mn, 0:66], lhsT=xt[:, m0:m0 + mn],
                                     rhs=w1t[:], start=True, stop=True)
                    st = wpool.tile([P, F], f32, tag="st")
                    nc.vector.tensor_copy(out=st[:mn, 0:66], in_=ps[:mn, 0:66])
                    nc.vector.memset(st[:mn, 66:68], 1.0)
                    nc.sync.dma_start(out=slab1[n0 + m0:n0 + m0 + mn, :],
                                      in_=st[:mn, :])

    cc1 = nc.alloc_semaphore("cc1")
    nc.gpsimd.collective_compute(
        "AllGather", mybir.AluOpType.bypass, replica_groups=rg,
        ins=[slab1.ap().opt()], outs=[table1.ap().opt()],
    ).then_inc(cc1, 1)
    nc.gpsimd.wait_ge(cc1, 1)

    _edge_phase(nc, meta, table1, ad1_dram, iota128, src_in, rel_in,
                layer=1, slab_out=slab2, ad_out=ad2_dram,
                b1_rep=b1_rep, w2s_rep=w2s_rep, w2d_rep=w2d_rep,
                ident_in=None, wfin=None, bfin_rep=None,
                x_out=None, logits_out=None)

    cc2 = nc.alloc_semaphore("cc2")
    nc.gpsimd.collective_compute(
        "AllGather", mybir.AluOpType.bypass, replica_groups=rg,
        ins=[slab2.ap().opt()], outs=[table2.ap().opt()],
    ).then_inc(cc2, 1)
    nc.gpsimd.wait_ge(cc2, 1)

    _edge_phase(nc, meta, table2, ad2_dram, iota128, src_in, rel_in,
                layer=2, slab_out=None, ad_out=None,
                b1_rep=None, w2s_rep=None, w2d_rep=None,
                ident_in=ident_in, wfin=wfin, bfin_rep=bfin_rep,
                x_out=x_out, logits_out=logits_out)

    nc.compile()
    return nc


def kernel(**inputs):
    _install_axon_shim()
    from concourse.bass_utils import run_bass_kernel_spmd

    ft = np.asarray(inputs["ft_list"], dtype=np.float32)
    adj = np.asarray(inputs["adj_tensor"])
    args = [ft, adj] + [np.asarray(inputs[k], dtype=np.float32) for k in
                        ("W1", "a_src1", "a_dst1", "b1",
                         "W2", "a_src2", "a_dst2", "b2", "clas_W", "clas_b")]
    meta, per_core = _preprocess(*args)
    nc = _build(meta)
    res = run_bass_kernel_spmd(nc, per_core, core_ids=list(range(NC)))
    logits = np.concatenate([res.results[c]["logits_out"] for c in range(NC)], axis=0)
    x = np.concatenate([res.results[c]["x_out"] for c in range(NC)], axis=0)
    return logits.astype(np.float32), x.astype(np.float32)
